# revision 23
# baseline (speedup 1.0000x reference)
"""Trainium2 Bass kernel for the ExpCloudMMD loss.

reference math (gamma = 0.5):
  t1 = mean_{j,k} exp(-g*||p_j - p_k||^2)            over [8192, 8192]
  t2 = 2/(Nx*Np) * sum_{i,j} exp(-g*||x_i - p_j||^2) over [32768, 8192]
  out = t1 - t2  (f32 scalar)

Strategy (8 cores, SPMD, no collectives):
  - t2: shard x rows 8-way; each core computes its 4096x8192 cross block.
  - t1: symmetric Gram; only diagonal + strict upper 2048x2048 super-blocks
    are computed (host doubles the upper sums); 160 (row-block, col-group)
    pairs dealt round-robin to the 8 cores via the per-core `pslhs` input
    (program identical across cores).
  - The exp *argument* p.x - g|x|^2 - g|p|^2 comes straight out of a K=68
    matmul (bf16 hi/lo 4-way split product + norm channels).
  - exp + row-sum is split across BOTH elementwise engines:
      * ACT stream: [128, 1536] PSUM groups -> activation(Exp, accum_out).
        (1536 = the widest double-buffered group that leaves 2 PSUM banks
        for the DVE stream; per-instr overhead = 172cyc bubble + 187ns
        accumulator read + ~57ns seq.)
      * DVE stream: [128, 512] PSUM groups -> custom op opP computes
        q = p(x)^4 (deg-3 poly of exp(x/512), 8 ALU stages) into an SBUF
        stage buffer; every <=8 opPs one opS op (7 squarings + fused
        accumulate) turns the stage into q^128 = exp(x) row-sums.
    The DVE share (~30%: 1024 cols of most cross j-blocks, a few whole
    j-blocks, 512 of each full t1 pair) balances ACT at ~1.085 ns/col vs
    DVE at ~2.48 ns/col.
  - Accumulator columns land in a [128, n_cols] SBUF tile, DMA'd out; the
    final tiny weighted reduction happens on the host in float64.
"""

import threading

import ml_dtypes
import numpy as np

import concourse.bass as bass  # noqa: F401
import concourse.mybir as mybir
import concourse.tile as tile
from concourse import bacc, bass_utils

bf16 = ml_dtypes.bfloat16

GAMMA = 0.5
NX, NP, D = 32768, 8192, 16
N_CORES = 8
XS = NX // N_CORES     # 4096 x rows per core
K = 68                 # 4*16 (hi/lo product blocks) + 2 + 2 norm channels

# t1 coarse-triangle schedule: for col-super-group g (2048 particles),
# the computed row-blocks are the 16*(g+1) blocks of super-rows 0..g,
# dealt round-robin (r % 8) to cores -> per-core counts 2,4,6,8.
T1_COUNTS = [2, 4, 6, 8]
N_T1_PAIRS = sum(T1_COUNTS)                    # 20 per core
PS_COLS = N_T1_PAIRS * 128                     # 2560 pslhs columns per core

GA = 1536      # ACT group width (3 PSUM banks; x2 buffers = 6 banks)
GD = 512       # DVE group width (1 PSUM bank; x2 buffers = 2 banks)
STAGE_W = 8192  # opS batch width (16 opP outputs)
STAGE_N = STAGE_W // GD
ACT_INPLACE = True  # exp writes back into its PSUM tile (no SBUF scratch)

# Cross j-blocks fully assigned to the DVE stream (the rest give DVE
# 1024 or 2048 of their 4096 cols). Spread evenly to keep both engines fed.
N_JB = NP // 128                               # 64
DVE_FULL_JB = frozenset({5, 16, 27, 38, 49, 60})
# dw=2048 blocks: ACT half is two 1024 groups (rebalance knob).
DVE_HALF_JB = frozenset({13, 46})


def _t1_pairs(core):
    """[(row_block, col_group, weight)] for this core, in program order."""
    pairs = []
    for g in range(4):
        rows = [r for r in range(16 * (g + 1)) if r % N_CORES == core]
        assert len(rows) == T1_COUNTS[g]
        for r in rows:
            pairs.append((r, g, 1.0 if r // 16 == g else 2.0))
    return pairs


def _t1_slots():
    """Per pslhs slot s: (col_group g, is_diag) — identical across cores.

    Within each g the first T1_COUNTS[g]-2 slots are strict-upper full
    pairs (weight 2); the last 2 slots are the two diagonal-super-block
    rows handled at 1024 granularity (weights [1,2] and [1])."""
    slots = []
    for g in range(4):
        for t in range(T1_COUNTS[g]):
            slots.append((g, t >= T1_COUNTS[g] - 2))
    return slots


# t1 full pairs (12) whose DVE slice widens to 1024 (tail balance knob).
# 0: the cross-zone's spilled final opS already balances the t1 zone.
T1_DW1024 = 0


def _plan():
    """The shared instruction schedule, consumed by both _build_nc and
    _col_meta.  Ops:
      ('act',  src, slot, cstart, width, kind, w)
      ('dve',  src, slot, cstart, kind, w)   # one [128, GD] opP group
      ('flush',)                             # force opS on current stage
    src is 'x' (plhs/xrhs) or 'p' (pslhs/prhs)."""
    ops = []
    # The 12 ACT-only diagonal-trio groups, interleaved into the cross
    # zone once prhs/pslhs have surely landed.
    diag = []
    for g in range(4):
        s0 = sum(T1_COUNTS[:g]) + T1_COUNTS[g] - 2
        base = g * 2048
        diag.append(("act", "p", s0, base, 1024, "t1", 1.0))
        diag.append(("act", "p", s0, base + 1024, 1024, "t1", 2.0))
        diag.append(("act", "p", s0 + 1, base + 1024, 1024, "t1", 1.0))
    diag_at = {24 + 3 * i: d for i, d in enumerate(diag)}

    for j in range(N_JB):
        if j == 0:
            # Prologue block: chunk-aligned 1024-wide ACT groups so the
            # first groups depend on exactly one xrhs DMA chunk each.
            ops.append(("dve", "x", j, 0, "t2", 1.0))
            ops.append(("dve", "x", j, 512, "t2", 1.0))
            ops.append(("act", "x", j, 1024, 1024, "t2", 1.0))
            ops.append(("act", "x", j, 2048, 1024, "t2", 1.0))
            ops.append(("act", "x", j, 3072, 1024, "t2", 1.0))
        elif j in DVE_FULL_JB:
            for q in range(XS // GD):
                ops.append(("dve", "x", j, q * GD, "t2", 1.0))
        elif j in DVE_HALF_JB:
            ops.append(("dve", "x", j, 0, "t2", 1.0))
            ops.append(("dve", "x", j, 512, "t2", 1.0))
            ops.append(("act", "x", j, 2048, 1024, "t2", 1.0))
            ops.append(("dve", "x", j, 1024, "t2", 1.0))
            ops.append(("dve", "x", j, 1536, "t2", 1.0))
            ops.append(("act", "x", j, 3072, 1024, "t2", 1.0))
        else:
            ops.append(("dve", "x", j, 0, "t2", 1.0))
            ops.append(("dve", "x", j, 512, "t2", 1.0))
            ops.append(("act", "x", j, 1024, GA, "t2", 1.0))
            ops.append(("act", "x", j, 1024 + GA, GA, "t2", 1.0))
        if j in diag_at:
            ops.append(diag_at[j])
    ops.append(("flush",))

    # t1 full pairs: first T1_DW1024 get a 1024 DVE slice, the rest 512.
    n_dve = 0
    for s, (g, is_diag) in enumerate(_t1_slots()):
        if is_diag:
            continue
        base = g * 2048
        wide = n_dve < 2 * T1_DW1024
        ops.append(("dve", "p", s, base, "t1", 2.0))
        n_dve += 1
        if wide:
            ops.append(("dve", "p", s, base + GD, "t1", 2.0))
            n_dve += 1
            ops.append(("act", "p", s, base + 1024, 1024, "t1", 2.0))
        else:
            ops.append(("act", "p", s, base + GD, GA, "t1", 2.0))
        if n_dve == 8:
            ops.append(("flush",))
    ops.append(("flush",))
    return ops


N_PCHUNK = 8  # plhs load chunks for early compute start


def _build_nc(repeats=1):
    nc = bacc.Bacc(
        "TRN2",
        target_bir_lowering=False,
        debug=False,
        enable_asserts=False,
        num_devices=N_CORES,
    )
    dt = mybir.dt
    plhs = nc.dram_tensor("plhs", [K, NP], dt.bfloat16, kind="ExternalInput").ap()
    prhs = nc.dram_tensor("prhs", [K, NP], dt.bfloat16, kind="ExternalInput").ap()
    xrhs = nc.dram_tensor("xrhs", [K, XS], dt.bfloat16, kind="ExternalInput").ap()
    pslhs = nc.dram_tensor("pslhs", [K, PS_COLS], dt.bfloat16, kind="ExternalInput").ap()
    n_cols = len(_col_meta())
    acc_d = nc.dram_tensor("acc", [128, n_cols], dt.float32, kind="ExternalOutput").ap()
    opP, opS = _register_dve_exp_ops()
    ct = [float(v) for v in _EXP_CT]

    with tile.TileContext(nc) as tc:
        with (
            tc.tile_pool(name="const", bufs=1) as const,
            tc.tile_pool(name="scrp", bufs=2) as scrp,
            tc.tile_pool(name="psp", bufs=2, space="PSUM") as psp,
            tc.tile_pool(name="psd", bufs=2, space="PSUM") as psd,
            tc.tile_pool(name="stagep", bufs=2) as stagep,
            tc.tile_pool(name="scr3p", bufs=2) as scr3p,
        ):
            sb_plhs = const.tile([K, NP], dt.bfloat16)
            sb_prhs = const.tile([K, NP], dt.bfloat16)
            sb_xrhs = const.tile([K, XS], dt.bfloat16)
            sb_pslhs = const.tile([K, PS_COLS], dt.bfloat16)
            sb_acc = const.tile([128, n_cols], dt.float32)
            sb_tiny = const.tile([1, 1], dt.float32)

            # Warm the ACT exp table set (~2.7us) during the DMA prologue.
            nc.gpsimd.memset(sb_tiny[:], 0.0)
            nc.scalar.activation(
                sb_tiny[:], sb_tiny[:], mybir.ActivationFunctionType.Exp
            )

            # Input loads, in consumption order.  The j=0 stationary rides
            # the (otherwise idle) Pool engine's queue so it lands in
            # parallel with the first xrhs chunk on the SP queue.
            pchunk = NP // N_PCHUNK
            # Parallel queues: Pool carries the j=0 stationary, DVE its own
            # upcoming stationaries, ACT the xrhs chunk its first group
            # reads; SP streams the rest.
            nc.gpsimd.dma_start(sb_plhs[:, 0:128], plhs[:, 0:128])
            nc.sync.dma_start(sb_xrhs[:, 0:1024], xrhs[:, 0:1024])
            nc.scalar.dma_start(sb_xrhs[:, 1024:2048], xrhs[:, 1024:2048])
            nc.sync.dma_start(sb_plhs[:, 128:pchunk], plhs[:, 128:pchunk])
            nc.sync.dma_start(sb_xrhs[:, 2048:3072], xrhs[:, 2048:3072])
            nc.sync.dma_start(sb_xrhs[:, 3072:4096], xrhs[:, 3072:4096])
            for i in range(1, N_PCHUNK):
                s = slice(i * pchunk, (i + 1) * pchunk)
                nc.sync.dma_start(sb_plhs[:, s], plhs[:, s])
            nc.sync.dma_start(sb_pslhs[:], pslhs[:])
            nc.sync.dma_start(sb_prhs[:], prhs[:])

            col = 0

            def act_group(lhs_tile, j, rhs_tile, cstart, width):
                """ACT group: width/512 matmuls + fused exp-rowsum."""
                nonlocal col
                ps_t = psp.tile([128, GA], dt.float32, tag="ps")
                q0 = 0
                while q0 < width:
                    q1 = min(q0 + 512, width)
                    nc.tensor.matmul(
                        ps_t[:, q0:q1],
                        lhs_tile[:, j * 128:(j + 1) * 128],
                        rhs_tile[:, cstart + q0: cstart + q1],
                    )
                    q0 = q1
                if ACT_INPLACE:
                    out_ap = ps_t[:, :width]
                else:
                    scr = scrp.tile([128, GA], dt.float32, tag="scr")
                    out_ap = scr[:, :width]
                nc.scalar.activation(
                    out_ap,
                    ps_t[:, :width],
                    mybir.ActivationFunctionType.Exp,
                    accum_out=sb_acc[:, col:col + 1],
                )
                col += 1

            dve_state = {"fill": 0, "stage": None}

            def dve_group(lhs_tile, j, rhs_tile, cstart):
                """DVE group: 1 matmul + opP into the stage buffer."""
                st = dve_state
                ps_t = psd.tile([128, GD], dt.float32, tag="pd")
                nc.tensor.matmul(
                    ps_t[:],
                    lhs_tile[:, j * 128:(j + 1) * 128],
                    rhs_tile[:, cstart:cstart + GD],
                )
                if st["fill"] == 0:
                    st["stage"] = stagep.tile([128, STAGE_W], dt.float32,
                                              tag="stage", name="stage")
                sl = st["stage"][:, st["fill"] * GD:(st["fill"] + 1) * GD]
                nc.vector._custom_dve(
                    opP, out=sl, in0=ps_t[:],
                    s0=ct[2], s1=ct[1], imm2=ct[0],
                )
                st["fill"] += 1

            def dve_flush():
                """opS over the filled stage prefix -> one accum column."""
                nonlocal col
                st = dve_state
                if st["fill"] == 0:
                    return
                w = st["fill"] * GD
                scr3 = scr3p.tile([128, STAGE_W], dt.bfloat16, tag="scr3")
                nc.vector._custom_dve(
                    opS, out=scr3[:, :w], in0=st["stage"][:, :w],
                    s0=0.0, s1=0.0,
                    accum_out=sb_acc[:, col:col + 1],
                )
                col += 1
                st["fill"] = 0

            def dve_maybe_flush():
                if dve_state["fill"] == STAGE_N:
                    dve_flush()

            if repeats == 0:  # timing-only baseline: I/O but no compute
                nc.gpsimd.memset(sb_acc[:], 0.0)
            for _ in range(repeats):  # repeats>1 is a timing-only variant
                col = 0
                for op in _plan():
                    if op[0] == "act":
                        _, src, slot, cstart, width, _k, _w = op
                        lhs = sb_plhs if src == "x" else sb_pslhs
                        rhs = sb_xrhs if src == "x" else sb_prhs
                        act_group(lhs, slot, rhs, cstart, width)
                    elif op[0] == "dve":
                        _, src, slot, cstart, _k, _w = op
                        lhs = sb_plhs if src == "x" else sb_pslhs
                        rhs = sb_xrhs if src == "x" else sb_prhs
                        dve_group(lhs, slot, rhs, cstart)
                        dve_maybe_flush()
                    else:
                        dve_flush()
                if repeats:
                    assert col == n_cols, (col, n_cols)

            nc.sync.dma_start(acc_d[:], sb_acc[:])

    nc.compile()
    return nc


def _col_meta():
    """Per accum column (kind, weight), in emission order of _build_nc."""
    cols = []
    fill = 0
    meta = None
    for op in _plan():
        if op[0] == "act":
            cols.append((op[5], op[6]))
        elif op[0] == "dve":
            meta = (op[4], op[5])
            fill += 1
            if fill == STAGE_N:
                cols.append(meta)
                fill = 0
        else:
            if fill:
                cols.append(meta)
            fill = 0
    assert fill == 0
    return cols


def _split_hi_lo(v):
    vh = v.astype(bf16)
    vl = (v - vh.astype(np.float32)).astype(bf16)
    return vh, vl


def _enc_lhsT(p):
    """p: [n, 16] f32 -> [K, n] bf16 stationary-side encoding."""
    n = p.shape[0]
    ph, pl = _split_hi_lo(np.ascontiguousarray(p, np.float32))
    p2 = (-GAMMA * (p.astype(np.float64) ** 2).sum(-1)).astype(np.float32)
    p2h, p2l = _split_hi_lo(p2)
    out = np.empty((K, n), bf16)
    out[0:16] = ph.T
    out[16:32] = pl.T
    out[32:48] = ph.T
    out[48:64] = pl.T
    out[64] = p2h
    out[65] = p2l
    out[66] = bf16(-GAMMA)
    out[67] = bf16(-GAMMA)
    return out


def _enc_rhs(u):
    """u: [n, 16] f32 -> [K, n] bf16 moving-side encoding."""
    n = u.shape[0]
    uh, ul = _split_hi_lo(np.ascontiguousarray(u, np.float32))
    u2 = ((u.astype(np.float64) ** 2).sum(-1)).astype(np.float32)
    u2h, u2l = _split_hi_lo(u2)
    out = np.empty((K, n), bf16)
    out[0:16] = uh.T
    out[16:32] = uh.T
    out[32:48] = ul.T
    out[48:64] = ul.T
    out[64] = bf16(1.0)
    out[65] = bf16(1.0)
    out[66] = u2h
    out[67] = u2l
    return out


# ---- DVE exp: exp(x) = p(x)^512, p = deg-3 fit of exp(x/512) ----
_DVE_M = 512.0


def _fit_exp_coeffs():
    """p(x) = 1 + c1*x + c2*x^2 + c3*x^3 ~= exp(x/512); returns [c1, c2, c3].
    The constant term is pinned to the DVE's hardware `One`, and the bias of
    the fp32 squaring chain is tuned out on a chi2(32)-like argument mix."""
    M = _DVE_M
    lo, hi = -110.0 / M, 0.1 / M
    k = np.arange(4000)
    y = (lo + hi) / 2 + (hi - lo) / 2 * np.cos((2 * k + 1) * np.pi / (2 * len(k)))
    V = np.vander(y, 3, increasing=True) * y[:, None]
    w = 1.0 / np.exp(y)
    q = np.linalg.lstsq(V * w[:, None], (np.exp(y) - 1.0) * w, rcond=None)[0]
    ct = q / (M ** (np.arange(3) + 1))

    def emu(x, scale):
        c1, c2, c3 = (ct * scale).astype(np.float32)
        x = x.astype(np.float32)
        p = (((x * c3 + c2) * x + c1) * x + np.float32(1.0)).astype(np.float32)
        s = p
        for _ in range(9):
            s = (s * s).astype(np.float32)
        return s

    rng = np.random.default_rng(1)
    d2 = (rng.standard_normal((400000, 16)) * np.sqrt(2)).astype(np.float32)
    args = -0.5 * (d2 ** 2).sum(1)
    ref = np.exp(args.astype(np.float64))

    def bias(scale):
        return (emu(args, scale).sum(dtype=np.float64) - ref.sum()) / ref.sum()

    g1, g2 = bias(1.0), bias(1.0001)
    lam = -g1 / ((g2 - g1) / 0.0001)
    return (ct * (1.0 + lam)).astype(np.float32)


_EXP_CT = _fit_exp_coeffs()
_dve_exp_ops = None


def _register_dve_exp_ops():
    """Define + register the 2 custom DVE ops (idempotent, in-process)."""
    global _dve_exp_ops
    if _dve_exp_ops is not None:
        return _dve_exp_ops
    from operator import add as _opadd

    import concourse.dve_ops as dom
    from concourse.dve_spec import (
        C0, C1, C2, One, Spec, Src0, _has_src1, lower as _dve_lower, sq,
    )
    from concourse.dve_uop import DveOpSpec

    def _sq(v, n):
        s = v.astype(np.float32)
        for _ in range(n):
            s = (s * s).astype(np.float32)
        return s

    specs = [
        # p4 = (((c3*x + c2)*x + c1)*x + 1)^4   (constant term = hw One)
        ("ANT_EXPP512_1", Spec(
            body=sq(sq((((Src0 * C0) + C1) * Src0 + C2) * Src0 + One)),
            reference=lambda in0, in1, c0, c1, c2: _sq(
                ((in0.astype(np.float32) * np.float32(c0) + np.float32(c1))
                 * in0 + np.float32(c2)) * in0 + np.float32(1.0), 2
            ),
        )),
    ]
    _s = Src0
    for _ in range(7):
        _s = sq(_s)
    specs.append(
        ("ANT_EXPS512", Spec(
            body=_s,
            accum=_opadd,
            accum_init=C0,
            reference=dom._ref_body_sum(lambda in0, in1, c0, c1, c2: _sq(in0, 7)),
        ))
    )

    ops = []
    for name, spec in specs:
        if name in dom._SUB_OPCODE_FOR_NAME:
            ops.append(next(o for o in dom.OPS if o.name == name))
            continue
        row = dom._CUSTOM_DVE_ROW_BASE + len(dom.OPS)
        assert row < 0x20, "custom DVE opcode rows exhausted"
        op = dom.DveOp(name, spec, subdim=False, uops_sha={})
        for ver in ("v3", "v4"):
            u = _dve_lower(spec, ver=ver)
            sha = DveOpSpec(
                name=name, opcode=row, uops=u, rd1_en=_has_src1(spec)
            ).sha(ver)
            op.uops_sha[ver] = sha
        dom.OPS.append(op)
        dom._SUB_OPCODE_FOR_NAME[name] = row
        dom.CUSTOM_DVE_SPECS[name] = spec
        ops.append(op)
    _dve_exp_ops = tuple(ops)
    return _dve_exp_ops


_lock = threading.Lock()
_cached_nc = None


def _get_nc():
    global _cached_nc
    with _lock:
        if _cached_nc is None:
            _cached_nc = _build_nc()
        return _cached_nc


def _make_in_maps(x, particles):
    plhs = _enc_lhsT(particles)
    prhs = _enc_rhs(particles)
    in_maps = []
    for c in range(N_CORES):
        pairs = _t1_pairs(c)
        pslhs = np.concatenate(
            [plhs[:, r * 128:(r + 1) * 128] for r, _, _ in pairs], axis=1
        )
        in_maps.append(
            {
                "plhs": plhs,
                "prhs": prhs,
                "xrhs": _enc_rhs(x[c * XS:(c + 1) * XS]),
                "pslhs": np.ascontiguousarray(pslhs),
            }
        )
    return in_maps


def _combine(results):
    meta = _col_meta()
    t2_sum = 0.0
    t1_sum = 0.0
    for r in results:
        acc = r["acc"].astype(np.float64)
        s = acc.sum(axis=0)
        for i, (kind, w) in enumerate(meta):
            if kind == "t2":
                t2_sum += s[i]
            else:
                t1_sum += w * s[i]
    t1 = t1_sum / (float(NP) * NP)
    t2 = 2.0 * t2_sum / (float(NX) * NP)
    return np.float32(t1 - t2)


def kernel(x, particles):
    x = np.asarray(x, np.float32)
    particles = np.asarray(particles, np.float32)
    assert x.shape == (NX, D) and particles.shape == (NP, D)

    nc = _get_nc()
    in_maps = _make_in_maps(x, particles)
    res = bass_utils.run_bass_kernel_spmd(nc, in_maps, core_ids=list(range(N_CORES)))
    return _combine(res.results)


# revision 36
# speedup vs baseline: 1.2729x; 1.2729x over previous
"""Trainium2 Bass kernel for the ExpCloudMMD loss.

reference math (gamma = 0.5):
  t1 = mean_{j,k} exp(-g*||p_j - p_k||^2)            over [8192, 8192]
  t2 = 2/(Nx*Np) * sum_{i,j} exp(-g*||x_i - p_j||^2) over [32768, 8192]
  out = t1 - t2  (f32 scalar)

Strategy (8 cores, SPMD, no collectives):
  - t2: shard x rows 8-way; each core computes its 4096x8192 cross block.
  - t1: symmetric Gram; only diagonal + strict upper 2048x2048 super-blocks
    are computed (host doubles the upper sums); 160 (row-block, col-group)
    pairs dealt round-robin to the 8 cores via the per-core `pslhs` input
    (program identical across cores).
  - The exp *argument* p.x - g|x|^2 - g|p|^2 comes straight out of a K=68
    matmul (bf16 hi/lo 4-way split product + norm channels).
  - exp + row-sum is split across BOTH elementwise engines:
      * ACT stream: [128, 1536] PSUM groups -> activation(Exp, accum_out).
        (1536 = the widest double-buffered group that leaves 2 PSUM banks
        for the DVE stream; per-instr overhead = 172cyc bubble + 187ns
        accumulator read + ~57ns seq.)
      * DVE stream: [128, 512] PSUM groups -> custom op opP computes
        q = p(x)^4 (deg-3 poly of exp(x/512), 8 ALU stages) into an SBUF
        stage buffer; every <=8 opPs one opS op (7 squarings + fused
        accumulate) turns the stage into q^128 = exp(x) row-sums.
    The DVE share (~30%: 1024 cols of most cross j-blocks, a few whole
    j-blocks, 512 of each full t1 pair) balances ACT at ~1.085 ns/col vs
    DVE at ~2.48 ns/col.
  - Accumulator columns land in a [128, n_cols] SBUF tile, DMA'd out; the
    final tiny weighted reduction happens on the host in float64.
"""

import threading

import ml_dtypes
import numpy as np

import concourse.bass as bass  # noqa: F401
import concourse.mybir as mybir
import concourse.tile as tile
from concourse import bacc, bass_utils

bf16 = ml_dtypes.bfloat16

GAMMA = 0.5
NX, NP, D = 32768, 8192, 16
N_CORES = 8
XS = NX // N_CORES     # 4096 x rows per core
K = 68                 # 4*16 (hi/lo product blocks) + 2 + 2 norm channels

# t1 coarse-triangle schedule: for col-super-group g (2048 particles),
# the computed row-blocks are the 16*(g+1) blocks of super-rows 0..g,
# dealt round-robin (r % 8) to cores -> per-core counts 2,4,6,8.
T1_COUNTS = [2, 4, 6, 8]
N_T1_PAIRS = sum(T1_COUNTS)                    # 20 per core
PS_COLS = N_T1_PAIRS * 128                     # 2560 pslhs columns per core

GA = 1536      # ACT group width (3 PSUM banks; x2 buffers = 6 banks)
GD = 512       # DVE group width (1 PSUM bank; x2 buffers = 2 banks)
STAGE_W = 8192  # opS batch width (16 opP outputs)
STAGE_N = STAGE_W // GD
ACT_INPLACE = True  # exp writes back into its PSUM tile (no SBUF scratch)

# Cross j-blocks fully assigned to the DVE stream (the rest give DVE
# 1024 or 2048 of their 4096 cols). Spread evenly to keep both engines fed.
N_JB = NP // 128                               # 64
DVE_FULL_JB = frozenset({5, 16, 27, 38, 49, 60})
# dw=2048 blocks: ACT half is two 1024 groups (rebalance knob).
DVE_HALF_JB = frozenset({13, 46})


def _t1_pairs(core):
    """[(row_block, col_group, weight)] for this core, in program order."""
    pairs = []
    for g in range(4):
        rows = [r for r in range(16 * (g + 1)) if r % N_CORES == core]
        assert len(rows) == T1_COUNTS[g]
        for r in rows:
            pairs.append((r, g, 1.0 if r // 16 == g else 2.0))
    return pairs


def _t1_slots():
    """Per pslhs slot s: (col_group g, is_diag) — identical across cores.

    Within each g the first T1_COUNTS[g]-2 slots are strict-upper full
    pairs (weight 2); the last 2 slots are the two diagonal-super-block
    rows handled at 1024 granularity (weights [1,2] and [1])."""
    slots = []
    for g in range(4):
        for t in range(T1_COUNTS[g]):
            slots.append((g, t >= T1_COUNTS[g] - 2))
    return slots


# t1 full pairs (12) whose DVE slice widens to 1024 (tail balance knob).
# 0: the cross-zone's spilled final opS already balances the t1 zone.
T1_DW1024 = 0


def _plan():
    """The shared instruction schedule, consumed by both _build_nc and
    _col_meta.  Ops:
      ('act',  src, slot, cstart, width, kind, w)
      ('dve',  src, slot, cstart, kind, w)   # one [128, GD] opP group
      ('flush',)                             # force opS on current stage
    src is 'x' (plhs/xrhs) or 'p' (pslhs/prhs)."""
    ops = []
    # The 12 ACT-only diagonal-trio groups, interleaved into the cross
    # zone once prhs/pslhs have surely landed.
    diag = []
    for g in range(4):
        s0 = sum(T1_COUNTS[:g]) + T1_COUNTS[g] - 2
        base = g * 2048
        diag.append(("act", "p", s0, base, 1024, "t1", 1.0))
        diag.append(("act", "p", s0, base + 1024, 1024, "t1", 2.0))
        diag.append(("act", "p", s0 + 1, base + 1024, 1024, "t1", 1.0))
    diag_at = {24 + 3 * i: d for i, d in enumerate(diag)}

    for j in range(N_JB):
        if j == 0:
            # Prologue block: chunk-aligned ACT groups so the first groups
            # depend on the smallest possible prefix of xrhs DMA chunks.
            ops.append(("dve", "x", j, 0, "t2", 1.0))
            ops.append(("dve", "x", j, 512, "t2", 1.0))
            ops.append(("act", "x", j, 1024, 1024, "t2", 1.0))
            ops.append(("act", "x", j, 2048, 1024, "t2", 1.0))
            ops.append(("act", "x", j, 3072, 1024, "t2", 1.0))
        elif j in DVE_FULL_JB:
            for q in range(XS // GD):
                ops.append(("dve", "x", j, q * GD, "t2", 1.0))
        elif j in DVE_HALF_JB:
            ops.append(("dve", "x", j, 0, "t2", 1.0))
            ops.append(("dve", "x", j, 512, "t2", 1.0))
            ops.append(("act", "x", j, 2048, 1024, "t2", 1.0))
            ops.append(("dve", "x", j, 1024, "t2", 1.0))
            ops.append(("dve", "x", j, 1536, "t2", 1.0))
            ops.append(("act", "x", j, 3072, 1024, "t2", 1.0))
        else:
            ops.append(("dve", "x", j, 0, "t2", 1.0))
            ops.append(("dve", "x", j, 512, "t2", 1.0))
            ops.append(("act", "x", j, 1024, GA, "t2", 1.0))
            ops.append(("act", "x", j, 1024 + GA, GA, "t2", 1.0))
        if j in diag_at:
            ops.append(diag_at[j])
    ops.append(("flush",))

    # t1 full pairs: first T1_DW1024 get a 1024 DVE slice, the rest 512.
    n_dve = 0
    for s, (g, is_diag) in enumerate(_t1_slots()):
        if is_diag:
            continue
        base = g * 2048
        wide = n_dve < 2 * T1_DW1024
        ops.append(("dve", "p", s, base, "t1", 2.0))
        n_dve += 1
        if wide:
            ops.append(("dve", "p", s, base + GD, "t1", 2.0))
            n_dve += 1
            ops.append(("act", "p", s, base + 1024, 1024, "t1", 2.0))
        else:
            ops.append(("act", "p", s, base + GD, GA, "t1", 2.0))
        if n_dve == 8:
            ops.append(("flush",))
    ops.append(("flush",))
    return ops


N_PCHUNK = 8  # plhs load chunks for early compute start


def _build_nc(repeats=1, act_inplace=None, ops_accum=True, only=None):
    if act_inplace is None:
        act_inplace = ACT_INPLACE
    nc = bacc.Bacc(
        "TRN2",
        target_bir_lowering=False,
        debug=False,
        enable_asserts=False,
        num_devices=N_CORES,
    )
    dt = mybir.dt
    plhs = nc.dram_tensor("plhs", [K, NP], dt.bfloat16, kind="ExternalInput").ap()
    prhs = nc.dram_tensor("prhs", [K, NP], dt.bfloat16, kind="ExternalInput").ap()
    xrhs = nc.dram_tensor("xrhs", [K, XS], dt.bfloat16, kind="ExternalInput").ap()
    pslhs = nc.dram_tensor("pslhs", [K, PS_COLS], dt.bfloat16, kind="ExternalInput").ap()
    n_cols = len(_col_meta())
    acc_d = nc.dram_tensor("acc", [128, n_cols], dt.float32, kind="ExternalOutput").ap()
    opP, opS = _register_dve_exp_ops()
    ct = [float(v) for v in _EXP_CT]

    with tile.TileContext(nc) as tc:
        with (
            tc.tile_pool(name="const", bufs=1) as const,
            tc.tile_pool(name="scrp", bufs=2) as scrp,
            tc.tile_pool(name="psp", bufs=2, space="PSUM") as psp,
            tc.tile_pool(name="psd", bufs=2, space="PSUM") as psd,
            tc.tile_pool(name="stagep", bufs=2) as stagep,
            tc.tile_pool(name="scr3p", bufs=2) as scr3p,
        ):
            sb_plhs = const.tile([K, NP], dt.bfloat16)
            sb_prhs = const.tile([K, NP], dt.bfloat16)
            sb_xrhs = const.tile([K, XS], dt.bfloat16)
            sb_pslhs = const.tile([K, PS_COLS], dt.bfloat16)
            sb_acc = const.tile([128, n_cols], dt.float32)
            sb_tiny = const.tile([1, 1], dt.float32)

            # Input loads, in consumption order.  Parallel queues: Pool
            # carries the j=0 stationary, ACT the xrhs chunk its first
            # group reads (issued BEFORE the table-load warmup so the DMA
            # starts immediately); SP streams the rest.
            pchunk = NP // N_PCHUNK
            nc.gpsimd.dma_start(sb_plhs[:, 0:128], plhs[:, 0:128])
            nc.sync.dma_start(sb_xrhs[:, 0:1024], xrhs[:, 0:1024])
            nc.scalar.dma_start(sb_xrhs[:, 1024:2048], xrhs[:, 1024:2048])
            nc.sync.dma_start(sb_plhs[:, 128:pchunk], plhs[:, 128:pchunk])
            nc.sync.dma_start(sb_xrhs[:, 2048:3072], xrhs[:, 2048:3072])
            nc.sync.dma_start(sb_xrhs[:, 3072:4096], xrhs[:, 3072:4096])

            # Warm the ACT exp table set (~2.7us) during the DMA prologue.
            nc.gpsimd.memset(sb_tiny[:], 0.0)
            nc.scalar.activation(
                sb_tiny[:], sb_tiny[:], mybir.ActivationFunctionType.Exp
            )
            for i in range(1, N_PCHUNK):
                s = slice(i * pchunk, (i + 1) * pchunk)
                nc.sync.dma_start(sb_plhs[:, s], plhs[:, s])
            nc.sync.dma_start(sb_pslhs[:], pslhs[:])
            nc.sync.dma_start(sb_prhs[:], prhs[:])

            col = 0

            def act_group(lhs_tile, j, rhs_tile, cstart, width):
                """ACT group: width/512 matmuls + fused exp-rowsum."""
                nonlocal col
                ps_t = psp.tile([128, GA], dt.float32, tag="ps")
                q0 = 0
                while q0 < width:
                    q1 = min(q0 + 512, width)
                    nc.tensor.matmul(
                        ps_t[:, q0:q1],
                        lhs_tile[:, j * 128:(j + 1) * 128],
                        rhs_tile[:, cstart + q0: cstart + q1],
                    )
                    q0 = q1
                if act_inplace:
                    out_ap = ps_t[:, :width]
                else:
                    scr = scrp.tile([128, GA], dt.float32, tag="scr")
                    out_ap = scr[:, :width]
                nc.scalar.activation(
                    out_ap,
                    ps_t[:, :width],
                    mybir.ActivationFunctionType.Exp,
                    accum_out=sb_acc[:, col:col + 1],
                )
                col += 1

            dve_state = {"fill": 0, "stage": None}

            def dve_group(lhs_tile, j, rhs_tile, cstart):
                """DVE group: 1 matmul + opP into the stage buffer."""
                st = dve_state
                ps_t = psd.tile([128, GD], dt.float32, tag="pd")
                nc.tensor.matmul(
                    ps_t[:],
                    lhs_tile[:, j * 128:(j + 1) * 128],
                    rhs_tile[:, cstart:cstart + GD],
                )
                if st["fill"] == 0:
                    st["stage"] = stagep.tile([128, STAGE_W], dt.float32,
                                              tag="stage", name="stage")
                sl = st["stage"][:, st["fill"] * GD:(st["fill"] + 1) * GD]
                nc.vector._custom_dve(
                    opP, out=sl, in0=ps_t[:],
                    s0=ct[2], s1=ct[1], imm2=ct[0],
                )
                st["fill"] += 1

            def dve_flush():
                """opS over the filled stage prefix -> one accum column."""
                nonlocal col
                st = dve_state
                if st["fill"] == 0:
                    return
                w = st["fill"] * GD
                scr3 = scr3p.tile([128, STAGE_W], dt.bfloat16, tag="scr3")
                nc.vector._custom_dve(
                    opS, out=scr3[:, :w], in0=st["stage"][:, :w],
                    s0=0.0, s1=0.0,
                    accum_out=sb_acc[:, col:col + 1] if ops_accum else None,
                )
                col += 1
                st["fill"] = 0

            def dve_maybe_flush():
                if dve_state["fill"] == STAGE_N:
                    dve_flush()

            if repeats == 0 or not ops_accum or only:  # timing-only variants
                nc.gpsimd.memset(sb_acc[:], 0.0)
            for _ in range(repeats):  # repeats>1 is a timing-only variant
                col = 0
                for op in _plan():
                    if op[0] == "act":
                        if only == "dve":
                            continue
                        _, src, slot, cstart, width, _k, _w = op
                        lhs = sb_plhs if src == "x" else sb_pslhs
                        rhs = sb_xrhs if src == "x" else sb_prhs
                        act_group(lhs, slot, rhs, cstart, width)
                    elif op[0] == "dve":
                        if only == "act":
                            continue
                        _, src, slot, cstart, _k, _w = op
                        lhs = sb_plhs if src == "x" else sb_pslhs
                        rhs = sb_xrhs if src == "x" else sb_prhs
                        dve_group(lhs, slot, rhs, cstart)
                        dve_maybe_flush()
                    else:
                        if only != "act":
                            dve_flush()
                if repeats and not only:
                    assert col == n_cols, (col, n_cols)

            nc.sync.dma_start(acc_d[:], sb_acc[:])

    nc.compile()
    return nc


def _col_meta():
    """Per accum column (kind, weight), in emission order of _build_nc."""
    cols = []
    fill = 0
    meta = None
    for op in _plan():
        if op[0] == "act":
            cols.append((op[5], op[6]))
        elif op[0] == "dve":
            meta = (op[4], op[5])
            fill += 1
            if fill == STAGE_N:
                cols.append(meta)
                fill = 0
        else:
            if fill:
                cols.append(meta)
            fill = 0
    assert fill == 0
    return cols


def _split_hi_lo(v):
    vh = v.astype(bf16)
    vl = (v - vh.astype(np.float32)).astype(bf16)
    return vh, vl


def _enc_lhsT(p):
    """p: [n, 16] f32 -> [K, n] bf16 stationary-side encoding."""
    n = p.shape[0]
    ph, pl = _split_hi_lo(np.ascontiguousarray(p, np.float32))
    p2 = (-GAMMA * (p.astype(np.float64) ** 2).sum(-1)).astype(np.float32)
    p2h, p2l = _split_hi_lo(p2)
    out = np.empty((K, n), bf16)
    out[0:16] = ph.T
    out[16:32] = pl.T
    out[32:48] = ph.T
    out[48:64] = pl.T
    out[64] = p2h
    out[65] = p2l
    out[66] = bf16(-GAMMA)
    out[67] = bf16(-GAMMA)
    return out


def _enc_rhs(u):
    """u: [n, 16] f32 -> [K, n] bf16 moving-side encoding."""
    n = u.shape[0]
    uh, ul = _split_hi_lo(np.ascontiguousarray(u, np.float32))
    u2 = ((u.astype(np.float64) ** 2).sum(-1)).astype(np.float32)
    u2h, u2l = _split_hi_lo(u2)
    out = np.empty((K, n), bf16)
    out[0:16] = uh.T
    out[16:32] = uh.T
    out[32:48] = ul.T
    out[48:64] = ul.T
    out[64] = bf16(1.0)
    out[65] = bf16(1.0)
    out[66] = u2h
    out[67] = u2l
    return out


# ---- DVE exp: exp(x) = p(x)^512, p = deg-3 fit of exp(x/512) ----
_DVE_M = 512.0


def _fit_exp_coeffs():
    """p(x) = 1 + c1*x + c2*x^2 + c3*x^3 ~= exp(x/512); returns [c1, c2, c3].
    The constant term is pinned to the DVE's hardware `One`, and the bias of
    the fp32 squaring chain is tuned out on a chi2(32)-like argument mix."""
    M = _DVE_M
    lo, hi = -110.0 / M, 0.1 / M
    k = np.arange(4000)
    y = (lo + hi) / 2 + (hi - lo) / 2 * np.cos((2 * k + 1) * np.pi / (2 * len(k)))
    V = np.vander(y, 3, increasing=True) * y[:, None]
    w = 1.0 / np.exp(y)
    q = np.linalg.lstsq(V * w[:, None], (np.exp(y) - 1.0) * w, rcond=None)[0]
    ct = q / (M ** (np.arange(3) + 1))

    def emu(x, scale):
        c1, c2, c3 = (ct * scale).astype(np.float32)
        x = x.astype(np.float32)
        p = (((x * c3 + c2) * x + c1) * x + np.float32(1.0)).astype(np.float32)
        s = p
        for _ in range(9):
            s = (s * s).astype(np.float32)
        return s

    rng = np.random.default_rng(1)
    d2 = (rng.standard_normal((400000, 16)) * np.sqrt(2)).astype(np.float32)
    args = -0.5 * (d2 ** 2).sum(1)
    ref = np.exp(args.astype(np.float64))

    def bias(scale):
        return (emu(args, scale).sum(dtype=np.float64) - ref.sum()) / ref.sum()

    g1, g2 = bias(1.0), bias(1.0001)
    lam = -g1 / ((g2 - g1) / 0.0001)
    return (ct * (1.0 + lam)).astype(np.float32)


_EXP_CT = _fit_exp_coeffs()
_dve_exp_ops = None


def _register_dve_exp_ops():
    """Define + register the 2 custom DVE ops (idempotent, in-process)."""
    global _dve_exp_ops
    if _dve_exp_ops is not None:
        return _dve_exp_ops
    from operator import add as _opadd

    import concourse.dve_ops as dom
    from concourse.dve_spec import (
        C0, C1, C2, One, Spec, Src0, _has_src1, lower as _dve_lower, sq,
    )
    from concourse.dve_uop import DveOpSpec

    def _sq(v, n):
        s = v.astype(np.float32)
        for _ in range(n):
            s = (s * s).astype(np.float32)
        return s

    specs = [
        # p4 = (((c3*x + c2)*x + c1)*x + 1)^4   (constant term = hw One)
        ("ANT_EXPP512_1", Spec(
            body=sq(sq((((Src0 * C0) + C1) * Src0 + C2) * Src0 + One)),
            reference=lambda in0, in1, c0, c1, c2: _sq(
                ((in0.astype(np.float32) * np.float32(c0) + np.float32(c1))
                 * in0 + np.float32(c2)) * in0 + np.float32(1.0), 2
            ),
        )),
    ]
    _s = Src0
    for _ in range(7):
        _s = sq(_s)
    specs.append(
        ("ANT_EXPS512", Spec(
            body=_s,
            accum=_opadd,
            accum_init=C0,
            reference=dom._ref_body_sum(lambda in0, in1, c0, c1, c2: _sq(in0, 7)),
        ))
    )

    ops = []
    for name, spec in specs:
        if name in dom._SUB_OPCODE_FOR_NAME:
            ops.append(next(o for o in dom.OPS if o.name == name))
            continue
        row = dom._CUSTOM_DVE_ROW_BASE + len(dom.OPS)
        assert row < 0x20, "custom DVE opcode rows exhausted"
        op = dom.DveOp(name, spec, subdim=False, uops_sha={})
        for ver in ("v3", "v4"):
            u = _dve_lower(spec, ver=ver)
            sha = DveOpSpec(
                name=name, opcode=row, uops=u, rd1_en=_has_src1(spec)
            ).sha(ver)
            op.uops_sha[ver] = sha
        dom.OPS.append(op)
        dom._SUB_OPCODE_FOR_NAME[name] = row
        dom.CUSTOM_DVE_SPECS[name] = spec
        ops.append(op)
    _dve_exp_ops = tuple(ops)
    return _dve_exp_ops


_lock = threading.Lock()
_cached_nc = None


def _get_nc():
    global _cached_nc
    with _lock:
        if _cached_nc is None:
            _cached_nc = _build_nc()
        return _cached_nc


def _make_in_maps(x, particles):
    plhs = _enc_lhsT(particles)
    prhs = _enc_rhs(particles)
    in_maps = []
    for c in range(N_CORES):
        pairs = _t1_pairs(c)
        pslhs = np.concatenate(
            [plhs[:, r * 128:(r + 1) * 128] for r, _, _ in pairs], axis=1
        )
        in_maps.append(
            {
                "plhs": plhs,
                "prhs": prhs,
                "xrhs": _enc_rhs(x[c * XS:(c + 1) * XS]),
                "pslhs": np.ascontiguousarray(pslhs),
            }
        )
    return in_maps


def _combine(results):
    meta = _col_meta()
    t2_sum = 0.0
    t1_sum = 0.0
    for r in results:
        acc = r["acc"].astype(np.float64)
        s = acc.sum(axis=0)
        for i, (kind, w) in enumerate(meta):
            if kind == "t2":
                t2_sum += s[i]
            else:
                t1_sum += w * s[i]
    t1 = t1_sum / (float(NP) * NP)
    t2 = 2.0 * t2_sum / (float(NX) * NP)
    return np.float32(t1 - t2)


def kernel(x, particles):
    x = np.asarray(x, np.float32)
    particles = np.asarray(particles, np.float32)
    assert x.shape == (NX, D) and particles.shape == (NP, D)

    nc = _get_nc()
    in_maps = _make_in_maps(x, particles)
    res = bass_utils.run_bass_kernel_spmd(nc, in_maps, core_ids=list(range(N_CORES)))
    return _combine(res.results)


# revision 49
# speedup vs baseline: 1.4797x; 1.1625x over previous
"""Trainium2 Bass kernel for the ExpCloudMMD loss.

reference math (gamma = 0.5):
  t1 = mean_{j,k} exp(-g*||p_j - p_k||^2)            over [8192, 8192]
  t2 = 2/(Nx*Np) * sum_{i,j} exp(-g*||x_i - p_j||^2) over [32768, 8192]
  out = t1 - t2  (f32 scalar)

Strategy (8 cores, SPMD, no collectives):
  - t2: shard x rows 8-way; each core computes its 4096x8192 cross block.
  - t1: symmetric Gram; only diagonal + strict upper 2048x2048 super-blocks
    are computed (host doubles the upper sums); 160 (row-block, col-group)
    pairs dealt round-robin to the 8 cores via the per-core `pslhs` input
    (program identical across cores).
  - The exp *argument* p.x - g|x|^2 - g|p|^2 comes straight out of a K=68
    matmul (bf16 hi/lo 4-way split product + norm channels).
  - exp + row-sum is split across BOTH elementwise engines:
      * ACT stream: [128, 1536] PSUM groups -> activation(Exp, accum_out),
        writing the (unused) exp values back in place to PSUM, which is
        the cheapest legal destination.  1536 = the widest double-buffered
        group that leaves 2 PSUM banks for the DVE stream (PSUM pool tiles
        are bank-quantized, so 6+2 banks is the only viable split); ACT
        per-instr overhead = 172cyc PSUM bubble + 187ns accumulator read.
      * DVE stream: [128, 512] PSUM groups -> custom op opP computes
        q = p(x)^4 (deg-3 poly of exp(x/512), 8 ALU stages) into an SBUF
        stage buffer; every <=16 opPs one opS op (7 squarings + fused
        accumulate, whose seed state is a single cycle - NOT a second
        stream pass) turns the stage into q^128 = exp(x) row-sums.
    The DVE share (~31%: 2560 cols of 13 cross j-blocks, 1024 of the
    other 50, 512 of each full t1 pair) balances ACT at ~1.05 ns/col
    against DVE at ~2.33 ns/col; TimelineSim models both engines >93%
    busy, 228.6us total vs 314us for the ACT-only baseline.  Whole
    j-blocks on the DVE (an earlier layout, 226.6us modeled) are avoided
    on purpose: they idle the PE for ~4us at a time, beyond the ~3us
    p-state ramp threshold, which produced bimodal 210-330us HW timings;
    this layout caps PE idle gaps at ~1.3us.
  - Prologue: DMA chunks ride four queues (SP/ACT/Pool) ordered so the
    first DVE matmul and first ACT group start ~3-4us in; j=0 is split
    into chunk-aligned groups to match DMA arrival.
  - Accumulator columns land in a [128, n_cols] SBUF tile, DMA'd out; the
    final tiny weighted reduction happens on the host in float64.
"""

import threading

import ml_dtypes
import numpy as np

import concourse.bass as bass  # noqa: F401
import concourse.mybir as mybir
import concourse.tile as tile
from concourse import bacc, bass_utils

bf16 = ml_dtypes.bfloat16

GAMMA = 0.5
NX, NP, D = 32768, 8192, 16
N_CORES = 8
XS = NX // N_CORES     # 4096 x rows per core
K = 68                 # 4*16 (hi/lo product blocks) + 2 + 2 norm channels

# t1 coarse-triangle schedule: for col-super-group g (2048 particles),
# the computed row-blocks are the 16*(g+1) blocks of super-rows 0..g,
# dealt round-robin (r % 8) to cores -> per-core counts 2,4,6,8.
T1_COUNTS = [2, 4, 6, 8]
N_T1_PAIRS = sum(T1_COUNTS)                    # 20 per core
PS_COLS = N_T1_PAIRS * 128                     # 2560 pslhs columns per core

GA = 1536      # ACT group width (3 PSUM banks; x2 buffers = 6 banks)
GD = 512       # DVE group width (1 PSUM bank; x2 buffers = 2 banks)
STAGE_W = 8192  # opS batch width (16 opP outputs)
STAGE_N = STAGE_W // GD
ACT_INPLACE = True  # exp writes back into its PSUM tile (no SBUF scratch)

# Cross j-blocks fully assigned to the DVE stream (the rest give DVE
# 1024 or 2048 of their 4096 cols). Spread evenly to keep both engines fed.
N_JB = NP // 128                               # 64
DVE_FULL_JB = frozenset()
# dw=2048 blocks: ACT half is two 1024 groups (rebalance knob).
DVE_HALF_JB = frozenset()
# dw=2560 blocks: DVE gets [0:2560] (5 opPs), ACT one full 1536 group.
# Every cross ACT group is then a maximally-efficient 1536-wide
# instruction, and unlike full-DVE blocks the PE never idles >~1.5us,
# keeping it out of p-state drops (suspected cause of bimodal HW timings
# with the full-DVE layout).
DVE_C_JB = frozenset(set(range(4, 64, 5)) | {62})


def _t1_pairs(core):
    """[(row_block, col_group, weight)] for this core, in program order."""
    pairs = []
    for g in range(4):
        rows = [r for r in range(16 * (g + 1)) if r % N_CORES == core]
        assert len(rows) == T1_COUNTS[g]
        for r in rows:
            pairs.append((r, g, 1.0 if r // 16 == g else 2.0))
    return pairs


def _t1_slots():
    """Per pslhs slot s: (col_group g, is_diag) — identical across cores.

    Within each g the first T1_COUNTS[g]-2 slots are strict-upper full
    pairs (weight 2); the last 2 slots are the two diagonal-super-block
    rows handled at 1024 granularity (weights [1,2] and [1])."""
    slots = []
    for g in range(4):
        for t in range(T1_COUNTS[g]):
            slots.append((g, t >= T1_COUNTS[g] - 2))
    return slots


# t1 full pairs (12) whose DVE slice widens to 1024 (tail balance knob).
# 0: the cross-zone's spilled final opS already balances the t1 zone.
T1_DW1024 = 0


def _plan():
    """The shared instruction schedule, consumed by both _build_nc and
    _col_meta.  Ops:
      ('act',  src, slot, cstart, width, kind, w)
      ('dve',  src, slot, cstart, kind, w)   # one [128, GD] opP group
      ('flush',)                             # force opS on current stage
    src is 'x' (plhs/xrhs) or 'p' (pslhs/prhs)."""
    ops = []
    # The 12 ACT-only diagonal-trio groups, interleaved into the cross
    # zone once prhs/pslhs have surely landed.
    diag = []
    for g in range(4):
        s0 = sum(T1_COUNTS[:g]) + T1_COUNTS[g] - 2
        base = g * 2048
        diag.append(("act", "p", s0, base, 1024, "t1", 1.0))
        diag.append(("act", "p", s0, base + 1024, 1024, "t1", 2.0))
        diag.append(("act", "p", s0 + 1, base + 1024, 1024, "t1", 1.0))
    diag_at = {24 + 3 * i: d for i, d in enumerate(diag)}

    for j in range(N_JB):
        if j == 0:
            # Prologue block: chunk-aligned ACT groups so the first groups
            # depend on the smallest possible prefix of xrhs DMA chunks.
            ops.append(("dve", "x", j, 0, "t2", 1.0))
            ops.append(("dve", "x", j, 512, "t2", 1.0))
            ops.append(("act", "x", j, 1024, 1024, "t2", 1.0))
            ops.append(("act", "x", j, 2048, 1024, "t2", 1.0))
            ops.append(("act", "x", j, 3072, 1024, "t2", 1.0))
        elif j in DVE_FULL_JB:
            for q in range(XS // GD):
                ops.append(("dve", "x", j, q * GD, "t2", 1.0))
        elif j in DVE_C_JB:
            ops.append(("dve", "x", j, 0, "t2", 1.0))
            ops.append(("dve", "x", j, 512, "t2", 1.0))
            ops.append(("act", "x", j, 2560, GA, "t2", 1.0))
            ops.append(("dve", "x", j, 1024, "t2", 1.0))
            ops.append(("dve", "x", j, 1536, "t2", 1.0))
            ops.append(("dve", "x", j, 2048, "t2", 1.0))
        elif j in DVE_HALF_JB:
            ops.append(("dve", "x", j, 0, "t2", 1.0))
            ops.append(("dve", "x", j, 512, "t2", 1.0))
            ops.append(("act", "x", j, 2048, 1024, "t2", 1.0))
            ops.append(("dve", "x", j, 1024, "t2", 1.0))
            ops.append(("dve", "x", j, 1536, "t2", 1.0))
            ops.append(("act", "x", j, 3072, 1024, "t2", 1.0))
        else:
            ops.append(("dve", "x", j, 0, "t2", 1.0))
            ops.append(("dve", "x", j, 512, "t2", 1.0))
            ops.append(("act", "x", j, 1024, GA, "t2", 1.0))
            ops.append(("act", "x", j, 1024 + GA, GA, "t2", 1.0))
        if j in diag_at:
            ops.append(diag_at[j])
    ops.append(("flush",))

    # t1 full pairs: first T1_DW1024 get a 1024 DVE slice, the rest 512.
    # The LAST pair goes entirely to ACT (512+1536 groups) so the DVE
    # stream drains in step with ACT's tail instead of 2us after it.
    full_slots = [s for s, (_g, d) in enumerate(_t1_slots()) if not d]
    n_dve = 0
    for s, (g, is_diag) in enumerate(_t1_slots()):
        if is_diag:
            continue
        base = g * 2048
        if s == full_slots[-1]:
            ops.append(("act", "p", s, base, 512, "t1", 2.0))
            ops.append(("act", "p", s, base + GD, GA, "t1", 2.0))
            continue
        wide = n_dve < 2 * T1_DW1024
        ops.append(("dve", "p", s, base, "t1", 2.0))
        n_dve += 1
        if wide:
            ops.append(("dve", "p", s, base + GD, "t1", 2.0))
            n_dve += 1
            ops.append(("act", "p", s, base + 1024, 1024, "t1", 2.0))
        else:
            ops.append(("act", "p", s, base + GD, GA, "t1", 2.0))
        if n_dve == 8:
            ops.append(("flush",))
    ops.append(("flush",))
    return ops


N_PCHUNK = 8  # plhs load chunks for early compute start


def _build_nc(repeats=1, act_inplace=None, ops_accum=True, only=None):
    if act_inplace is None:
        act_inplace = ACT_INPLACE
    nc = bacc.Bacc(
        "TRN2",
        target_bir_lowering=False,
        debug=False,
        enable_asserts=False,
        num_devices=N_CORES,
    )
    dt = mybir.dt
    plhs = nc.dram_tensor("plhs", [K, NP], dt.bfloat16, kind="ExternalInput").ap()
    prhs = nc.dram_tensor("prhs", [K, NP], dt.bfloat16, kind="ExternalInput").ap()
    xrhs = nc.dram_tensor("xrhs", [K, XS], dt.bfloat16, kind="ExternalInput").ap()
    pslhs = nc.dram_tensor("pslhs", [K, PS_COLS], dt.bfloat16, kind="ExternalInput").ap()
    n_cols = len(_col_meta())
    acc_d = nc.dram_tensor("acc", [128, n_cols], dt.float32, kind="ExternalOutput").ap()
    opP, opS = _register_dve_exp_ops()
    ct = [float(v) for v in _EXP_CT]

    with tile.TileContext(nc) as tc:
        with (
            tc.tile_pool(name="const", bufs=1) as const,
            tc.tile_pool(name="scrp", bufs=2) as scrp,
            tc.tile_pool(name="psp", bufs=2, space="PSUM") as psp,
            tc.tile_pool(name="psd", bufs=2, space="PSUM") as psd,
            tc.tile_pool(name="stagep", bufs=2) as stagep,
            tc.tile_pool(name="scr3p", bufs=2) as scr3p,
        ):
            sb_plhs = const.tile([K, NP], dt.bfloat16)
            sb_prhs = const.tile([K, NP], dt.bfloat16)
            sb_xrhs = const.tile([K, XS], dt.bfloat16)
            sb_pslhs = const.tile([K, PS_COLS], dt.bfloat16)
            sb_acc = const.tile([128, n_cols], dt.float32)
            sb_tiny = const.tile([1, 1], dt.float32)

            # Input loads, in consumption order.  Parallel queues: Pool
            # carries the j=0 stationary, ACT the xrhs chunk its first
            # group reads (issued BEFORE the table-load warmup so the DMA
            # starts immediately); SP streams the rest.
            pchunk = NP // N_PCHUNK
            nc.gpsimd.dma_start(sb_plhs[:, 0:128], plhs[:, 0:128])
            nc.sync.dma_start(sb_xrhs[:, 0:1024], xrhs[:, 0:1024])
            nc.scalar.dma_start(sb_xrhs[:, 1024:2048], xrhs[:, 1024:2048])
            nc.sync.dma_start(sb_plhs[:, 128:pchunk], plhs[:, 128:pchunk])
            nc.sync.dma_start(sb_xrhs[:, 2048:3072], xrhs[:, 2048:3072])
            nc.sync.dma_start(sb_xrhs[:, 3072:4096], xrhs[:, 3072:4096])

            # Warm the ACT exp table set (~2.7us) during the DMA prologue.
            nc.gpsimd.memset(sb_tiny[:], 0.0)
            nc.scalar.activation(
                sb_tiny[:], sb_tiny[:], mybir.ActivationFunctionType.Exp
            )
            for i in range(1, N_PCHUNK):
                s = slice(i * pchunk, (i + 1) * pchunk)
                nc.sync.dma_start(sb_plhs[:, s], plhs[:, s])
            nc.sync.dma_start(sb_pslhs[:], pslhs[:])
            nc.sync.dma_start(sb_prhs[:], prhs[:])

            col = 0

            def act_group(lhs_tile, j, rhs_tile, cstart, width):
                """ACT group: width/512 matmuls + fused exp-rowsum."""
                nonlocal col
                ps_t = psp.tile([128, GA], dt.float32, tag="ps")
                q0 = 0
                while q0 < width:
                    q1 = min(q0 + 512, width)
                    nc.tensor.matmul(
                        ps_t[:, q0:q1],
                        lhs_tile[:, j * 128:(j + 1) * 128],
                        rhs_tile[:, cstart + q0: cstart + q1],
                    )
                    q0 = q1
                if act_inplace:
                    out_ap = ps_t[:, :width]
                else:
                    scr = scrp.tile([128, GA], dt.float32, tag="scr")
                    out_ap = scr[:, :width]
                nc.scalar.activation(
                    out_ap,
                    ps_t[:, :width],
                    mybir.ActivationFunctionType.Exp,
                    accum_out=sb_acc[:, col:col + 1],
                )
                col += 1

            dve_state = {"fill": 0, "stage": None}

            def dve_group(lhs_tile, j, rhs_tile, cstart):
                """DVE group: 1 matmul + opP into the stage buffer."""
                st = dve_state
                ps_t = psd.tile([128, GD], dt.float32, tag="pd")
                nc.tensor.matmul(
                    ps_t[:],
                    lhs_tile[:, j * 128:(j + 1) * 128],
                    rhs_tile[:, cstart:cstart + GD],
                )
                if st["fill"] == 0:
                    st["stage"] = stagep.tile([128, STAGE_W], dt.float32,
                                              tag="stage", name="stage")
                sl = st["stage"][:, st["fill"] * GD:(st["fill"] + 1) * GD]
                nc.vector._custom_dve(
                    opP, out=sl, in0=ps_t[:],
                    s0=ct[2], s1=ct[1], imm2=ct[0],
                )
                st["fill"] += 1

            def dve_flush():
                """opS over the filled stage prefix -> one accum column."""
                nonlocal col
                st = dve_state
                if st["fill"] == 0:
                    return
                w = st["fill"] * GD
                scr3 = scr3p.tile([128, STAGE_W], dt.bfloat16, tag="scr3")
                nc.vector._custom_dve(
                    opS, out=scr3[:, :w], in0=st["stage"][:, :w],
                    s0=0.0, s1=0.0,
                    accum_out=sb_acc[:, col:col + 1] if ops_accum else None,
                )
                col += 1
                st["fill"] = 0

            def dve_maybe_flush():
                if dve_state["fill"] == STAGE_N:
                    dve_flush()

            if repeats == 0 or not ops_accum or only:  # timing-only variants
                nc.gpsimd.memset(sb_acc[:], 0.0)
            for _ in range(repeats):  # repeats>1 is a timing-only variant
                col = 0
                for op in _plan():
                    if op[0] == "act":
                        if only == "dve":
                            continue
                        _, src, slot, cstart, width, _k, _w = op
                        lhs = sb_plhs if src == "x" else sb_pslhs
                        rhs = sb_xrhs if src == "x" else sb_prhs
                        act_group(lhs, slot, rhs, cstart, width)
                    elif op[0] == "dve":
                        if only == "act":
                            continue
                        _, src, slot, cstart, _k, _w = op
                        lhs = sb_plhs if src == "x" else sb_pslhs
                        rhs = sb_xrhs if src == "x" else sb_prhs
                        dve_group(lhs, slot, rhs, cstart)
                        dve_maybe_flush()
                    else:
                        if only != "act":
                            dve_flush()
                if repeats and not only:
                    assert col == n_cols, (col, n_cols)

            nc.sync.dma_start(acc_d[:], sb_acc[:])

    nc.compile()
    return nc


def _col_meta():
    """Per accum column (kind, weight), in emission order of _build_nc."""
    cols = []
    fill = 0
    meta = None
    for op in _plan():
        if op[0] == "act":
            cols.append((op[5], op[6]))
        elif op[0] == "dve":
            meta = (op[4], op[5])
            fill += 1
            if fill == STAGE_N:
                cols.append(meta)
                fill = 0
        else:
            if fill:
                cols.append(meta)
            fill = 0
    assert fill == 0
    return cols


def _split_hi_lo(v):
    vh = v.astype(bf16)
    vl = (v - vh.astype(np.float32)).astype(bf16)
    return vh, vl


def _enc_lhsT(p):
    """p: [n, 16] f32 -> [K, n] bf16 stationary-side encoding."""
    n = p.shape[0]
    ph, pl = _split_hi_lo(np.ascontiguousarray(p, np.float32))
    p2 = (-GAMMA * (p.astype(np.float64) ** 2).sum(-1)).astype(np.float32)
    p2h, p2l = _split_hi_lo(p2)
    out = np.empty((K, n), bf16)
    out[0:16] = ph.T
    out[16:32] = pl.T
    out[32:48] = ph.T
    out[48:64] = pl.T
    out[64] = p2h
    out[65] = p2l
    out[66] = bf16(-GAMMA)
    out[67] = bf16(-GAMMA)
    return out


def _enc_rhs(u):
    """u: [n, 16] f32 -> [K, n] bf16 moving-side encoding."""
    n = u.shape[0]
    uh, ul = _split_hi_lo(np.ascontiguousarray(u, np.float32))
    u2 = ((u.astype(np.float64) ** 2).sum(-1)).astype(np.float32)
    u2h, u2l = _split_hi_lo(u2)
    out = np.empty((K, n), bf16)
    out[0:16] = uh.T
    out[16:32] = uh.T
    out[32:48] = ul.T
    out[48:64] = ul.T
    out[64] = bf16(1.0)
    out[65] = bf16(1.0)
    out[66] = u2h
    out[67] = u2l
    return out


# ---- DVE exp: exp(x) = p(x)^512, p = deg-3 fit of exp(x/512) ----
_DVE_M = 512.0


def _fit_exp_coeffs():
    """p(x) = 1 + c1*x + c2*x^2 + c3*x^3 ~= exp(x/512); returns [c1, c2, c3].
    The constant term is pinned to the DVE's hardware `One`, and the bias of
    the fp32 squaring chain is tuned out on a chi2(32)-like argument mix."""
    M = _DVE_M
    lo, hi = -110.0 / M, 0.1 / M
    k = np.arange(4000)
    y = (lo + hi) / 2 + (hi - lo) / 2 * np.cos((2 * k + 1) * np.pi / (2 * len(k)))
    V = np.vander(y, 3, increasing=True) * y[:, None]
    w = 1.0 / np.exp(y)
    q = np.linalg.lstsq(V * w[:, None], (np.exp(y) - 1.0) * w, rcond=None)[0]
    ct = q / (M ** (np.arange(3) + 1))

    def emu(x, scale):
        c1, c2, c3 = (ct * scale).astype(np.float32)
        x = x.astype(np.float32)
        p = (((x * c3 + c2) * x + c1) * x + np.float32(1.0)).astype(np.float32)
        s = p
        for _ in range(9):
            s = (s * s).astype(np.float32)
        return s

    rng = np.random.default_rng(1)
    d2 = (rng.standard_normal((400000, 16)) * np.sqrt(2)).astype(np.float32)
    args = -0.5 * (d2 ** 2).sum(1)
    ref = np.exp(args.astype(np.float64))

    def bias(scale):
        return (emu(args, scale).sum(dtype=np.float64) - ref.sum()) / ref.sum()

    g1, g2 = bias(1.0), bias(1.0001)
    lam = -g1 / ((g2 - g1) / 0.0001)
    return (ct * (1.0 + lam)).astype(np.float32)


_EXP_CT = _fit_exp_coeffs()
_dve_exp_ops = None


def _register_dve_exp_ops():
    """Define + register the 2 custom DVE ops (idempotent, in-process)."""
    global _dve_exp_ops
    if _dve_exp_ops is not None:
        return _dve_exp_ops
    from operator import add as _opadd

    import concourse.dve_ops as dom
    from concourse.dve_spec import (
        C0, C1, C2, One, Spec, Src0, _has_src1, lower as _dve_lower, sq,
    )
    from concourse.dve_uop import DveOpSpec

    def _sq(v, n):
        s = v.astype(np.float32)
        for _ in range(n):
            s = (s * s).astype(np.float32)
        return s

    specs = [
        # p4 = (((c3*x + c2)*x + c1)*x + 1)^4   (constant term = hw One)
        ("ANT_EXPP512_1", Spec(
            body=sq(sq((((Src0 * C0) + C1) * Src0 + C2) * Src0 + One)),
            reference=lambda in0, in1, c0, c1, c2: _sq(
                ((in0.astype(np.float32) * np.float32(c0) + np.float32(c1))
                 * in0 + np.float32(c2)) * in0 + np.float32(1.0), 2
            ),
        )),
    ]
    _s = Src0
    for _ in range(7):
        _s = sq(_s)
    specs.append(
        ("ANT_EXPS512", Spec(
            body=_s,
            accum=_opadd,
            accum_init=C0,
            reference=dom._ref_body_sum(lambda in0, in1, c0, c1, c2: _sq(in0, 7)),
        ))
    )

    ops = []
    for name, spec in specs:
        if name in dom._SUB_OPCODE_FOR_NAME:
            ops.append(next(o for o in dom.OPS if o.name == name))
            continue
        row = dom._CUSTOM_DVE_ROW_BASE + len(dom.OPS)
        assert row < 0x20, "custom DVE opcode rows exhausted"
        op = dom.DveOp(name, spec, subdim=False, uops_sha={})
        for ver in ("v3", "v4"):
            u = _dve_lower(spec, ver=ver)
            sha = DveOpSpec(
                name=name, opcode=row, uops=u, rd1_en=_has_src1(spec)
            ).sha(ver)
            op.uops_sha[ver] = sha
        dom.OPS.append(op)
        dom._SUB_OPCODE_FOR_NAME[name] = row
        dom.CUSTOM_DVE_SPECS[name] = spec
        ops.append(op)
    _dve_exp_ops = tuple(ops)
    return _dve_exp_ops


_lock = threading.Lock()
_cached_nc = None


def _get_nc():
    global _cached_nc
    with _lock:
        if _cached_nc is None:
            _cached_nc = _build_nc()
        return _cached_nc


def _make_in_maps(x, particles):
    plhs = _enc_lhsT(particles)
    prhs = _enc_rhs(particles)
    in_maps = []
    for c in range(N_CORES):
        pairs = _t1_pairs(c)
        pslhs = np.concatenate(
            [plhs[:, r * 128:(r + 1) * 128] for r, _, _ in pairs], axis=1
        )
        in_maps.append(
            {
                "plhs": plhs,
                "prhs": prhs,
                "xrhs": _enc_rhs(x[c * XS:(c + 1) * XS]),
                "pslhs": np.ascontiguousarray(pslhs),
            }
        )
    return in_maps


def _combine(results):
    meta = _col_meta()
    t2_sum = 0.0
    t1_sum = 0.0
    for r in results:
        acc = r["acc"].astype(np.float64)
        s = acc.sum(axis=0)
        for i, (kind, w) in enumerate(meta):
            if kind == "t2":
                t2_sum += s[i]
            else:
                t1_sum += w * s[i]
    t1 = t1_sum / (float(NP) * NP)
    t2 = 2.0 * t2_sum / (float(NX) * NP)
    return np.float32(t1 - t2)


def kernel(x, particles):
    x = np.asarray(x, np.float32)
    particles = np.asarray(particles, np.float32)
    assert x.shape == (NX, D) and particles.shape == (NP, D)

    nc = _get_nc()
    in_maps = _make_in_maps(x, particles)
    res = bass_utils.run_bass_kernel_spmd(nc, in_maps, core_ids=list(range(N_CORES)))
    return _combine(res.results)
